# revision 2
# baseline (speedup 1.0000x reference)
"""Trainium2 Bass kernel v2 for nn_BotAwareGAT (2-layer hetero GAT + MLP).

vs v1:
  - Layer 2 gathers h1elu (128 cols, 512B/edge) instead of h2 (512 cols,
    1280B/edge): the W2 projection is applied AFTER aggregation.  Table2
    rows are [h1elu(128) | 1.0 | s2src(8) | s2dst(8) | pad].
  - L2 aggregation per 128-edge section: lhsT = Q[e,(h,j)] (one-hot x q,
    built on DVE), rhs = G[:, 0:129] -> psum A[(h,j), msgs+z].  Normalize by
    8z, transpose per 16-dst group, project with W2 head blocks (accumulated
    over heads), transpose, scatter-add into acc2.
  - L2 dst-score gathers: 1 row per dst (NSLOT idx) + one-hot REP matmuls to
    broadcast to all round partitions (8x fewer gather descriptors).
  - Layer 1 unchanged from v1 (64-dst groups, DVE-weighted messages).
"""

import numpy as np
import ml_dtypes

N = 20000
NCORES = 8
ND = N // NCORES            # 2500 dst nodes per core
NSLOT = 2560                # padded ranks per core
NB = NSLOT // 128           # 20 blocks of 128 ranks
HEADS = 8
POISON = N
T1C = 256                   # [h1(128) | s1src(8) | s1dst(8) | pad]
T2C = 256                   # [h1elu(128) | one | s2src(8) | s2dst(8) | pad]
NCH = 8                     # sections per dma_gather call
NEG = 0.2
DEBUG = False
GD1 = 64                    # L1 dsts per group (2 rounds/section)
NG1 = NSLOT // GD1          # 40
GD2 = 16                    # L2 dsts per group (8 rounds/section)
NG2 = NSLOT // GD2          # 160

bf16 = ml_dtypes.bfloat16


def _scatter_calls():
    out = []
    c0 = 0
    while c0 < NB:
        n = min(8, NB - c0)
        out.append((c0, n * 128))
        c0 += n
    return out


# ----------------------------------------------------------------------------
# host-side schedule construction (pure integer/layout work)
# ----------------------------------------------------------------------------

def _wrap16(a):
    """[L] int -> [128, L//16] int16 (dma_gather/scatter index layout)."""
    w = a.reshape(-1, 16).T.astype(np.int16)
    return np.tile(w, (8, 1))


def _percore_csr(src, dst):
    percore = []
    for k in range(NCORES):
        sel = (dst >= k * ND) & (dst < (k + 1) * ND)
        s = src[sel].astype(np.int64)
        d = (dst[sel] - k * ND).astype(np.int64)
        deg = np.bincount(d, minlength=ND)
        order = np.argsort(-deg, kind="stable")
        o = np.argsort(d, kind="stable")
        s_sorted = s[o]
        starts = np.zeros(ND + 1, np.int64)
        np.cumsum(deg, out=starts[1:])
        percore.append((deg, order, s_sorted, starts))
    return percore


def _schedule(percore, gd):
    """ELL schedule with gd dsts/group, 128//gd rounds per 128-edge section."""
    rpc = 128 // gd
    ng = NSLOT // gd
    Rg = np.zeros(ng, np.int64)
    for g in range(ng):
        mx = 1
        lo, hi = g * gd, (g + 1) * gd
        for (deg, order, _, _) in percore:
            real_hi = min(hi, ND)
            if lo < real_hi:
                mx = max(mx, int(deg[order[lo:real_hi]].max()))
        Rg[g] = ((mx + rpc - 1) // rpc) * rpc
    Cg = Rg // rpc
    cbase = np.zeros(ng, np.int64)
    np.cumsum(Cg[:-1], out=cbase[1:])
    TC = int(Cg.sum())
    TCpad = ((TC + NCH - 1) // NCH) * NCH

    gidx_all, sdti_all = [], []
    for k in range(NCORES):
        deg, order, s_sorted, starts = percore[k]
        gidx = np.full(TCpad * 128, POISON, np.int64)
        if gd == 64:
            sdti = np.full(2 * NSLOT, POISON, np.int64)
        else:
            sdti = np.full(NSLOT, POISON, np.int64)
        for g in range(ng):
            base = cbase[g]
            for j in range(gd):
                rank = g * gd + j
                if rank < ND:
                    dd = int(order[rank])
                    if gd == 64:
                        sdti[g * 128 + j] = k * ND + dd
                        sdti[g * 128 + gd + j] = k * ND + dd
                    else:
                        sdti[rank] = k * ND + dd
                    dg = int(deg[dd])
                    if dg:
                        r = np.arange(dg)
                        pos = (base + r // rpc) * 128 + (r % rpc) * gd + j
                        gidx[pos] = s_sorted[starts[dd]:starts[dd] + dg]
        ncalls = TCpad // NCH
        gidx_all.append(_wrap16(gidx).reshape(128, ncalls, NCH * 8)
                        .transpose(1, 0, 2))
        sdti_all.append(_wrap16(sdti))

    return dict(Rg=Rg, cbase=cbase, TC=TC, TCpad=TCpad,
                gidx=np.stack(gidx_all), sdti=np.stack(sdti_all))


def _host_prep(inputs):
    """Layout transforms of the inputs + schedules (no float math)."""
    x = np.asarray(inputs["x"], np.float32)
    W1 = np.asarray(inputs["W1"], np.float32)
    a1s = np.asarray(inputs["a1_src"], np.float32)
    a1d = np.asarray(inputs["a1_dst"], np.float32)
    W2 = np.asarray(inputs["W2"], np.float32)
    a2s = np.asarray(inputs["a2_src"], np.float32)
    a2d = np.asarray(inputs["a2_dst"], np.float32)
    Wc1 = np.asarray(inputs["Wc1"], np.float32)
    Wc2 = np.asarray(inputs["Wc2"], np.float32)

    shared = {}
    shared["xT"] = np.ascontiguousarray(x.T.reshape(2, 128, N)).astype(bf16)
    shared["w1"] = np.ascontiguousarray(W1.reshape(2, 2, 128, 128)).astype(bf16)
    shared["w1t"] = np.ascontiguousarray(W1.transpose(0, 2, 1)).astype(bf16)
    A1s = np.zeros((2, 128, 8), np.float32)
    A1d = np.zeros((2, 128, 8), np.float32)
    for t in range(2):
        for h in range(8):
            A1s[t, h * 16:(h + 1) * 16, h] = a1s[t, h]
            A1d[t, h * 16:(h + 1) * 16, h] = a1d[t, h]
    shared["a1m"] = np.stack([A1s, A1d], 1).astype(bf16)          # [2,2,128,8]
    shared["w2"] = W2.astype(bf16)                                 # [2,128,512]
    shared["w2t"] = np.ascontiguousarray(
        W2.transpose(0, 2, 1).reshape(2, 4, 128, 128)).astype(bf16)
    A2s = np.zeros((2, 512, 8), np.float32)
    A2d = np.zeros((2, 512, 8), np.float32)
    for t in range(2):
        for h in range(8):
            A2s[t, h * 64:(h + 1) * 64, h] = a2s[t, h]
            A2d[t, h * 64:(h + 1) * 64, h] = a2d[t, h]
    shared["a2m"] = np.stack([A2s, A2d], 1).reshape(2, 2, 4, 128, 8).astype(bf16)
    shared["wc1"] = Wc1.astype(bf16)
    shared["wc2"] = Wc2.astype(bf16)

    # one-hot [128, 64]: row e -> col e%64 (L1 aggregation)
    S1 = np.zeros((128, GD1), np.float32)
    for e in range(128):
        S1[e, e % GD1] = 1.0
    shared["sconst1"] = S1.astype(bf16)
    # one-hot [128, 16]: row e -> col e%16 (L2 Q build)
    S2 = np.zeros((128, GD2), np.float32)
    for e in range(128):
        S2[e, e % GD2] = 1.0
    shared["sconst2"] = S2.astype(bf16)
    # REPB[b, r, p] = 1 iff r == 16*b + p%16 (partition-block broadcast)
    RB = np.zeros((8, 128, 128), np.float32)
    for b in range(8):
        for p in range(128):
            RB[b, 16 * b + p % 16, p] = 1.0
    shared["repb"] = RB.astype(bf16)
    ident = np.eye(128, dtype=np.float32)
    shared["ident"] = ident.astype(bf16)
    p1 = np.zeros((1, T1C), np.float32)
    p1[0, 128:136] = -600.0
    shared["poison1"] = p1.astype(bf16)
    p2 = np.zeros((1, T2C), np.float32)
    p2[0, 129:137] = -600.0
    shared["poison2"] = p2.astype(bf16)
    shared["ones"] = np.ones((128, 1), np.float32).astype(bf16)

    ei_a = np.asarray(inputs["edge_index_a"])
    ei_b = np.asarray(inputs["edge_index_b"])
    pc_a = _percore_csr(ei_a[0], ei_a[1])
    pc_b = _percore_csr(ei_b[0], ei_b[1])
    scheds = {}
    scheds["1a"] = _schedule(pc_a, GD1)
    scheds["1b"] = _schedule(pc_b, GD1)
    scheds["2a"] = _schedule(pc_a, GD2)
    scheds["2b"] = _schedule(pc_b, GD2)

    scat = {}
    for t, pc in (("a", pc_a), ("b", pc_b)):
        per = []
        for k in range(NCORES):
            deg, order, _, _ = pc[k]
            sc = np.arange(NSLOT, dtype=np.int64)
            sc[:ND] = order[np.arange(ND)]
            # rank -> local dst row; trash ranks keep identity (rows >= ND)
            per.append(_wrap16(sc))
        scat[t] = np.stack(per)

    per_core = []
    for k in range(NCORES):
        m = dict(shared)
        for key, s in scheds.items():
            m[f"gidx_{key}"] = s["gidx"][k]
            m[f"sdti_{key}"] = s["sdti"][k]
        m["scat_a"] = scat["a"][k]
        m["scat_b"] = scat["b"][k]
        per_core.append(m)
    return per_core, scheds


# ----------------------------------------------------------------------------
# device kernel
# ----------------------------------------------------------------------------

def _patch_queue_aware_lanes():
    """Queue-aware SWDGE DMA semaphore-lane assignment (queue q -> lanes
    {2q, 2q+1})."""
    import concourse.tile_sem_assignment as tsa
    import concourse.mybir as mybir
    if getattr(tsa, "_qaware_patched", False):
        return
    orig = tsa.TileClockTick._assign_tick

    def patched(self, inst):
        if (isinstance(inst, tsa.DMAInst)
                and inst.engine == mybir.EngineType.Pool
                and not isinstance(inst, tsa.bass_isa.UserSyncedRemoteDMADescs)):
            q = getattr(inst, "queue_num", 0) or 0
            tog = getattr(self, "_q_toggle", None)
            if tog is None:
                tog = self._q_toggle = {}
            self.next_sw_dma_idx = (q * 2 + tog.get(q, 0)) % self.swdge_sem_count
            tog[q] = 1 - tog.get(q, 0)
        return orig(self, inst)

    tsa.TileClockTick._assign_tick = patched
    tsa._qaware_patched = True


def _build_nc(scheds):
    import concourse.bacc as bacc
    import concourse.mybir as mybir
    import concourse.tile as tile

    _patch_queue_aware_lanes()

    BF = mybir.dt.bfloat16
    F32 = mybir.dt.float32
    I16 = mybir.dt.int16
    AF = mybir.ActivationFunctionType
    OP = mybir.AluOpType

    nc = bacc.Bacc("TRN2", target_bir_lowering=False, debug=False,
                   num_devices=NCORES, num_swdge_queues=4)

    def din(name, shape, dt=BF):
        return nc.dram_tensor(name, shape, dt, kind="ExternalInput").ap()

    NSC = NSLOT // 16

    xT = din("xT", [2, 128, N])
    w1 = din("w1", [2, 2, 128, 128])
    w1t = din("w1t", [2, 128, 256])
    a1m = din("a1m", [2, 2, 128, 8])
    w2 = din("w2", [2, 128, 512])
    w2t = din("w2t", [2, 4, 128, 128])
    a2m = din("a2m", [2, 2, 4, 128, 8])
    wc1 = din("wc1", [64, 32])
    wc2 = din("wc2", [32, 2])
    sconst1 = din("sconst1", [128, GD1])
    sconst2 = din("sconst2", [128, GD2])
    repb = din("repb", [8, 128, 128])
    ident = din("ident", [128, 128])
    poison1 = din("poison1", [1, T1C])
    poison2 = din("poison2", [1, T2C])
    ones_d = din("ones", [128, 1])
    gidx_d = {key: din(f"gidx_{key}",
                       [scheds[key]["TCpad"] // NCH, 128, NCH * 8], I16)
              for key in scheds}
    sdti_d = {key: din(f"sdti_{key}",
                       [128, (2 * NSLOT if key[0] == "1" else NSLOT) // 16],
                       I16)
              for key in scheds}
    scat_d = {t: din(f"scat_{t}", [128, NSC], I16) for t in "ab"}
    out = nc.dram_tensor("out", [ND, 2], F32, kind="ExternalOutput").ap()
    if DEBUG:
        dbg_sdt2a = nc.dram_tensor("dbg_sdt2a", [128, NG2, 8], F32,
                                   kind="ExternalOutput").ap()
        dbg_parks2a = nc.dram_tensor("dbg_parks2a", [128, NB, 64], F32,
                                     kind="ExternalOutput").ap()
        dbg_parks2b = nc.dram_tensor("dbg_parks2b", [128, NB, 64], F32,
                                     kind="ExternalOutput").ap()
        dbg_t2a = nc.dram_tensor("dbg_t2a", [256, 256], BF,
                                 kind="ExternalOutput").ap()
        dbg_acc2 = nc.dram_tensor("dbg_acc2", [NSLOT, 64], F32,
                                  kind="ExternalOutput").ap()
        dbg_pa = nc.dram_tensor("dbg_pa", [8, 128, 129], F32,
                                kind="ExternalOutput").ap()
        dbg_an = nc.dram_tensor("dbg_an", [8, 128, 128], F32,
                                kind="ExternalOutput").ap()
        dbg_p8 = nc.dram_tensor("dbg_p8", [128, 8, 128], F32,
                                kind="ExternalOutput").ap()
        dbg_B = nc.dram_tensor("dbg_B", [64, 128], F32,
                               kind="ExternalOutput").ap()

    with tile.TileContext(nc) as tc:
        with tc.tile_pool(name="dram", bufs=1, space="DRAM") as dpool, \
             tc.tile_pool(name="const", bufs=1) as cpool:

            table1 = {t: dpool.tile([N + 1, T1C], BF, tag=f"tb1{t}",
                                    name=f"table1{t}") for t in "ab"}
            table2 = {t: dpool.tile([N + 1, T2C], BF, tag=f"tb2{t}",
                                    name=f"table2{t}") for t in "ab"}
            acc1 = dpool.tile([NSLOT, 128], F32, tag="acc1")
            acc2 = dpool.tile([NSLOT, 64], F32, tag="acc2")
            h2sliceT = [dpool.tile([128, 1280], BF, tag="h2sT0",
                                   name="h2sT0"),
                        dpool.tile([128, ND - 1280], BF, tag="h2sT1",
                                   name="h2sT1")]
            h2fullT = [dpool.tile([NCORES, 128, 1280], BF, tag="h2fT0",
                                  name="h2fT0"),
                       dpool.tile([NCORES, 128, ND - 1280], BF, tag="h2fT1",
                                  name="h2fT1")]

            # ---- constants ----
            sc1_sb = cpool.tile([128, GD1], BF, tag="sc1", name="sc1")
            nc.sync.dma_start(sc1_sb[:], sconst1[:])
            sc2_sb = cpool.tile([128, GD2], BF, tag="sc2", name="sc2")
            nc.sync.dma_start(sc2_sb[:], sconst2[:])
            repb_sb = cpool.tile([128, 8, 128], BF, tag="repb", name="repbsb")
            nc.sync.dma_start(repb_sb[:], repb.rearrange("b r p -> r b p"))
            id_sb = cpool.tile([128, 128], BF, tag="idsb", name="idsb")
            nc.sync.dma_start(id_sb[:], ident[:])
            ones_sb = cpool.tile([128, 1], BF, tag="onessb", name="onessb")
            nc.sync.dma_start(ones_sb[:], ones_d[:])
            wc1_sb = cpool.tile([64, 32], BF)
            nc.sync.dma_start(wc1_sb[:], wc1[:])
            wc2_sb = cpool.tile([32, 2], BF)
            nc.sync.dma_start(wc2_sb[:], wc2[:])
            w2cat = {}
            for ti, t in enumerate("ab"):
                w2cat[t] = cpool.tile([128, 512], BF, tag=f"w2cat{t}",
                                      name=f"w2cat{t}")
                nc.sync.dma_start(w2cat[t][:], w2[ti])
            scat_sb = {}
            sdti_sb = {}
            for t in "ab":
                scat_sb[t] = cpool.tile([128, NSC], I16, tag=f"scat{t}",
                                        name=f"scatsb{t}")
                nc.sync.dma_start(scat_sb[t][:], scat_d[t][:])
                nc.sync.dma_start(table1[t][N:N + 1, :], poison1[:])
                nc.sync.dma_start(table2[t][N:N + 1, :], poison2[:])
            for key in scheds:
                nsc = (2 * NSLOT if key[0] == "1" else NSLOT) // 16
                sdti_sb[key] = cpool.tile([128, nsc], I16, tag=f"sdti{key}",
                                          name=f"sdtisb{key}")
                nc.sync.dma_start(sdti_sb[key][:], sdti_d[key][:])

            # ---- weight augmentation ----
            w1ab = [cpool.tile([128, 2, 144], BF, tag=f"w1ab{c}",
                                name=f"w1ab{c}") for c in range(2)]
            w2ab = cpool.tile([128, 2, 144], BF, tag="w2ab", name="w2ab")
            with tc.tile_pool(name="aug", bufs=2) as augp, \
                 tc.tile_pool(name="augps", bufs=2, space="PSUM") as augps:
                for ti, t in enumerate("ab"):
                    wa = [w1ab[c][:, ti, :] for c in range(2)]
                    for c in range(2):
                        nc.sync.dma_start(wa[c][:, 0:128], w1[ti, c])
                    for si in range(2):
                        a_sb = augp.tile([128, 8], BF, tag="a1sb")
                        nc.sync.dma_start(a_sb[:], a1m[ti, si])
                        w1t_sb = augp.tile([128, 256], BF, tag="w1tsb")
                        nc.sync.dma_start(w1t_sb[:], w1t[ti])
                        ps = augps.tile([8, 256], F32, tag="wsps")
                        nc.tensor.matmul(out=ps[:], lhsT=a_sb[:], rhs=w1t_sb[:],
                                         start=True, stop=True)
                        s8 = augp.tile([8, 256], BF, tag="ws8")
                        nc.vector.tensor_copy(out=s8[:], in_=ps[:])
                        for c in range(2):
                            tp = augps.tile([128, 8], BF, tag="wstp")
                            nc.tensor.transpose(out=tp[:],
                                                in_=s8[:, c * 128:(c + 1) * 128],
                                                identity=id_sb[0:8, 0:8])
                            nc.vector.tensor_copy(
                                out=wa[c][:, 128 + si * 8:136 + si * 8],
                                in_=tp[:])
                    w2a = w2ab[:, ti, :]
                    nc.sync.dma_start(w2a[:, 0:128], ident[:])
                    for si in range(2):
                        ps = augps.tile([8, 128], F32, tag="w2ps")
                        for c in range(4):
                            a_sb = augp.tile([128, 8], BF, tag="a2sb")
                            nc.sync.dma_start(a_sb[:], a2m[ti, si, c])
                            w2t_sb = augp.tile([128, 128], BF, tag="w2tsb")
                            nc.sync.dma_start(w2t_sb[:], w2t[ti, c])
                            nc.tensor.matmul(out=ps[:], lhsT=a_sb[:],
                                             rhs=w2t_sb[:],
                                             start=(c == 0), stop=(c == 3))
                        s8 = augp.tile([8, 128], BF, tag="w2s8")
                        nc.vector.tensor_copy(out=s8[:], in_=ps[:])
                        tp = augps.tile([128, 8], BF, tag="w2tp")
                        nc.tensor.transpose(out=tp[:], in_=s8[:],
                                            identity=id_sb[0:8, 0:8])
                        nc.vector.tensor_copy(out=w2a[:, 128 + si * 8:136 + si * 8],
                                              in_=tp[:])

            # ---- phase 1: layer-1 tables ----
            with tc.tile_pool(name="ph1", bufs=3) as p1p, \
                 tc.tile_pool(name="ph1ps", bufs=3, space="PSUM") as p1ps:
                xt_sb = [p1p.tile([128, N], BF, tag=f"xt{c}", name=f"xtsb{c}",
                                  bufs=1) for c in range(2)]
                for c in range(2):
                    nc.sync.dma_start(xt_sb[c][:], xT[c])
                ob = {}
                for i in range((N + 127) // 128):
                    lo = i * 128
                    m = min(128, N - lo)
                    psd = p1ps.tile([128, 2, 144], F32, tag="t1ps")
                    nc.tensor.matmul(out=psd[:m], lhsT=xt_sb[0][:, lo:lo + m],
                                     rhs=w1ab[0][:], start=True, stop=False)
                    nc.tensor.matmul(out=psd[:m], lhsT=xt_sb[1][:, lo:lo + m],
                                     rhs=w1ab[1][:], start=False, stop=True)
                    for ti, t in enumerate("ab"):
                        ps = psd[:, ti, :]
                        if m == 128:
                            bi = i % 4
                            if bi == 0:
                                ob[t] = p1p.tile([128, 4, 144], BF,
                                                 tag=f"t1o{t}", name=f"t1o{t}")
                            nc.scalar.copy(out=ob[t][:, bi, :], in_=ps[:, :])
                            if bi == 3:
                                nc.sync.dma_start(
                                    table1[t][lo - 384:lo + 128, 0:144]
                                    .rearrange("(i p) c -> p i c", p=128),
                                    ob[t][:])
                        else:
                            o = p1p.tile([128, 144], BF, tag="t1os")
                            nc.scalar.copy(out=o[:m], in_=ps[:m])
                            nc.sync.dma_start(table1[t][lo:lo + m, 0:144],
                                              o[:m])

            # zero accumulators
            with tc.tile_pool(name="zacc", bufs=1) as zaccp:
                zt = zaccp.tile([128, NB, 128], F32)
                nc.vector.memset(zt[:], 0.0)
                nc.sync.dma_start(acc1.rearrange("(a p) c -> p a c", p=128),
                                  zt[:])
                nc.sync.dma_start(acc2.rearrange("(a p) c -> p a c", p=128),
                                  zt[:, :, 0:64])

            # ---- layer-1 edge phase (v1 scheme, 64-dst groups) ----
            with tc.tile_pool(name="park1", bufs=1) as parkp, \
                 tc.tile_pool(name="eg1", bufs=4) as gp, \
                 tc.tile_pool(name="ew1", bufs=3) as wp, \
                 tc.tile_pool(name="es1", bufs=4) as sp, \
                 tc.tile_pool(name="ef1", bufs=2) as fp, \
                 tc.tile_pool(name="eps1", bufs=3, space="PSUM") as pp:
                parks = {t: parkp.tile([128, NB, 128], F32, tag=f"park{t}",
                                       name=f"park1{t}") for t in "ab"}
                gidx_sb = {}
                sdt = {}
                for t in "ab":
                    key = "1" + t
                    ncalls = scheds[key]["TCpad"] // NCH
                    gidx_sb[t] = fp.tile([128, ncalls, NCH * 8], I16,
                                         tag=f"gidx{t}", name=f"gidx1{t}",
                                         bufs=1)
                    nc.sync.dma_start(gidx_sb[t][:],
                                      gidx_d[key].rearrange("c p s -> p c s"))
                    sdt[t] = fp.tile([128, NG1, 128], BF, tag=f"sdt{t}",
                                     name=f"sdt1{t}", bufs=1)
                    tview = table1[t][:, 128:256]
                    sdone = 0
                    while sdone < 2 * NSLOT:
                        n = min(1024, 2 * NSLOT - sdone)
                        nc.gpsimd.dma_gather(
                            sdt[t][:, sdone // 128:(sdone + n) // 128, :],
                            tview,
                            sdti_sb[key][:, sdone // 16:(sdone + n) // 16],
                            n, n, 128, elem_step=T1C, queue_num=3)
                        sdone += n

                st = {t: dict(call=-1, G=None, pa=None) for t in "ab"}
                qctr = [0]

                def do_group1(t, g):
                    key = "1" + t
                    sched = scheds[key]
                    cg = int(sched["Rg"][g] // 2)
                    base = int(sched["cbase"][g])
                    s_ = st[t]
                    if g % 2 == 0:
                        s_["pa"] = pp.tile([128, 136], F32, tag=f"pa{t}",
                                           name=f"pa1{t}")
                    pa = s_["pa"]
                    row0 = GD1 * (g % 2)
                    done = 0
                    while done < cg:
                        seg = min(NCH - (base + done) % NCH, cg - done)
                        call = (base + done) // NCH
                        coff = (base + done) % NCH
                        if call != s_["call"]:
                            G = gp.tile([128, NCH, 256], BF, tag=f"G{t}",
                                        name=f"G1{t}")
                            nc.gpsimd.dma_gather(
                                G[:, :, :], table1[t][:],
                                gidx_sb[t][:, call, :],
                                NCH * 128, NCH * 128, 256,
                                queue_num=qctr[0] % 4)
                            qctr[0] += 1
                            s_["call"] = call
                            s_["G"] = G
                        G = s_["G"]
                        sl = slice(coff, coff + seg)
                        u = sp.tile([128, NCH, 8], F32, tag=f"u{t}",
                                    name=f"u1{t}")
                        nc.vector.tensor_tensor(
                            out=u[:, :seg, :], in0=G[:, sl, 128:136],
                            in1=sdt[t][:, g, 8:16][:, None, :].to_broadcast(
                                [128, seg, 8]),
                            op=OP.add)
                        phi = sp.tile([128, NCH, 8], F32, tag=f"phi{t}",
                                      name=f"phi1{t}")
                        nc.vector.scalar_tensor_tensor(
                            out=phi[:, :seg, :], in0=u[:, :seg, :], scalar=NEG,
                            in1=u[:, :seg, :], op0=OP.mult, op1=OP.max)
                        q = sp.tile([128, NCH, 8], BF, tag=f"q{t}",
                                    name=f"q1{t}")
                        nc.scalar.activation(out=q[:, :seg, :],
                                             in_=phi[:, :seg, :], func=AF.Exp)
                        W = wp.tile([128, NCH, 136], BF, tag=f"W{t}",
                                    name=f"W1{t}")
                        nc.vector.tensor_tensor(
                            out=W[:, :seg, 0:128].rearrange(
                                "p s (h c) -> p s h c", h=8),
                            in0=G[:, sl, 0:128].rearrange(
                                "p s (h c) -> p s h c", h=8),
                            in1=q[:, :seg, :, None].to_broadcast(
                                [128, seg, 8, 16]),
                            op=OP.mult)
                        nc.scalar.copy(out=W[:, :seg, 128:136],
                                       in_=q[:, :seg, :])
                        for s in range(seg):
                            cc = done + s
                            nc.tensor.matmul(
                                out=pa[row0:row0 + GD1, :],
                                lhsT=sc1_sb[:], rhs=W[:, s, :],
                                start=(cc == 0), stop=(cc == cg - 1),
                                skip_group_check=True)
                        done += seg
                    if g % 2 == 1:
                        mi = g // 2
                        z8 = sp.tile([128, 8], F32, tag=f"z8{t}",
                                     name=f"z81{t}")
                        nc.vector.tensor_scalar(
                            out=z8[:], in0=pa[:, 128:136], scalar1=1.0,
                            scalar2=1e-30, op0=OP.mult, op1=OP.max)
                        rz = sp.tile([128, 8], F32, tag=f"rz{t}",
                                     name=f"rz1{t}")
                        nc.vector.reciprocal(out=rz[:], in_=z8[:])
                        nc.vector.tensor_tensor(
                            out=parks[t][:, mi, :].rearrange(
                                "p (h c) -> p h c", h=8),
                            in0=pa[:, 0:128].rearrange("p (h c) -> p h c", h=8),
                            in1=rz[:].to_broadcast([128, 8, 16]),
                            op=OP.mult)

                for g in range(NG1):
                    for t in "ab":
                        do_group1(t, g)
                for t in "ab":
                    for (c0, nI) in _scatter_calls():
                        nc.gpsimd.dma_scatter_add(
                            acc1[:], parks[t][:, c0:c0 + nI // 128, :],
                            scat_sb[t][:, c0 * 8:c0 * 8 + nI // 16],
                            nI, nI, 128, queue_num=3)

            # ---- combine + ELU helper ----
            def elu_combine(src_ap, cols, tilepool, dst_write):
                ntile = (ND + 127) // 128
                i = 0
                while i < ntile:
                    nb = min(4, ntile - 1 - i) if (ND - i * 128) >= 512 else 1
                    if i + nb >= ntile and ND - i * 128 < nb * 128:
                        nb = 1
                    m = min(nb * 128, ND - i * 128)
                    lo = i * 128
                    a = tilepool.tile([128, 4, cols], F32, tag="ec_a")
                    av = a.rearrange("p i c -> p (i c)")[:, 0:nb * cols]
                    if m == nb * 128:
                        nc.sync.dma_start(
                            a[:, 0:nb, :],
                            src_ap[lo:lo + m, :].rearrange("(i p) c -> p i c",
                                                           p=128))
                    else:
                        nc.sync.dma_start(a[:m, 0, :], src_ap[lo:lo + m, :])
                    e = tilepool.tile([128, 4, cols], F32, tag="ec_e")
                    ev = e.rearrange("p i c -> p (i c)")
                    nc.scalar.activation(out=ev[:, 0:nb * cols], in_=av,
                                         func=AF.Exp, scale=0.5)
                    em1 = tilepool.tile([128, 4, cols], F32, tag="ec_em1")
                    e1v = em1.rearrange("p i c -> p (i c)")
                    nc.vector.tensor_scalar(out=e1v[:, 0:nb * cols],
                                            in0=ev[:, 0:nb * cols],
                                            scalar1=-1.0,
                                            scalar2=None, op0=OP.add)
                    xm = tilepool.tile([128, 4, cols], F32, tag="ec_xm")
                    xv = xm.rearrange("p i c -> p (i c)")
                    nc.vector.tensor_scalar(out=xv[:, 0:nb * cols], in0=av,
                                            scalar1=0.5,
                                            scalar2=None, op0=OP.mult)
                    mk = tilepool.tile([128, 4, cols], mybir.dt.uint8,
                                       tag="ec_mk")
                    mv = mk.rearrange("p i c -> p (i c)")
                    nc.vector.tensor_scalar(out=mv[:, 0:nb * cols], in0=av,
                                            scalar1=0.0,
                                            scalar2=None, op0=OP.is_gt)
                    h = tilepool.tile([128, 4, cols], BF, tag="ec_h")
                    hv = h.rearrange("p i c -> p (i c)")
                    nc.vector.select(out=hv[:, 0:nb * cols],
                                     mask=mv[:, 0:nb * cols],
                                     on_true=xv[:, 0:nb * cols],
                                     on_false=e1v[:, 0:nb * cols])
                    for bi in range(nb):
                        mm = min(128, ND - (i + bi) * 128)
                        dst_write(i + bi, (i + bi) * 128, mm, h[:, bi, :])
                    i += nb

            # L1 combine -> transposed slice
            with tc.tile_pool(name="elu1", bufs=4) as elup, \
                 tc.tile_pool(name="elu1ps", bufs=3, space="PSUM") as elups:
                def wr1(i, lo, m, h):
                    tps = elups.tile([128, 128], BF, tag="e_tp")
                    nc.tensor.transpose(out=tps[:, :m], in_=h[:m, :],
                                        identity=id_sb[:m, :m])
                    ht = elup.tile([128, 128], BF, tag="e_ht")
                    nc.scalar.copy(out=ht[:, :m], in_=tps[:, :m])
                    if lo < 1280:
                        nc.sync.dma_start(h2sliceT[0][:, lo:lo + m], ht[:, :m])
                    else:
                        nc.sync.dma_start(h2sliceT[1][:, lo - 1280:lo - 1280 + m],
                                          ht[:, :m])
                elu_combine(acc1[:, :], 128, elup, wr1)

            for ci in range(2):
                nc.gpsimd.collective_compute(
                    "AllGather", mybir.AluOpType.bypass,
                    replica_groups=[list(range(NCORES))],
                    ins=[h2sliceT[ci].opt()], outs=[h2fullT[ci].opt()])

            # ---- phase 4: layer-2 tables ----
            with tc.tile_pool(name="ph4", bufs=6) as p4p, \
                 tc.tile_pool(name="ph4ps", bufs=3, space="PSUM") as p4ps:
                h2t_sb = p4p.tile([128, NCORES, ND], BF, tag="h2t",
                                  name="h2tsb", bufs=1)
                nc.sync.dma_start(h2t_sb[:, :, 0:1280],
                                  h2fullT[0].rearrange("k p j -> p k j"))
                nc.sync.dma_start(h2t_sb[:, :, 1280:ND],
                                  h2fullT[1].rearrange("k p j -> p k j"))
                for k8 in range(NCORES):
                    ob4 = {}
                    pend = {"a": 0, "b": 0}
                    base4 = {"a": 0, "b": 0}
                    for j in range((ND + 127) // 128):
                        lo = j * 128
                        m = min(128, ND - lo)
                        row = k8 * ND + lo
                        lhs = h2t_sb[:, k8, lo:lo + m]
                        psd4 = p4ps.tile([128, 2, 144], F32, tag="t2ps")
                        nc.tensor.matmul(out=psd4[:m], lhsT=lhs,
                                         rhs=w2ab[:], start=True, stop=True)
                        for ti, t in enumerate("ab"):
                            ps = psd4[:, ti, :]
                            if m == 128:
                                if pend[t] == 0:
                                    ob4[t] = p4p.tile([128, 4, 145], BF,
                                                      tag=f"t2o{t}",
                                                      name=f"t2o{t}")
                                    base4[t] = row
                                bi = pend[t]
                                nc.scalar.copy(out=ob4[t][:, bi, 0:128],
                                               in_=ps[:, 0:128])
                                nc.vector.tensor_copy(out=ob4[t][:, bi, 128:129],
                                                      in_=ones_sb[:])
                                nc.vector.tensor_copy(out=ob4[t][:, bi, 129:145],
                                                      in_=ps[:, 128:144])
                                pend[t] += 1
                                if pend[t] == 4 or lo + 256 > ND:
                                    nn = pend[t]
                                    nc.sync.dma_start(
                                        table2[t][base4[t]:base4[t] + nn * 128,
                                                  0:145]
                                        .rearrange("(i p) c -> p i c", p=128),
                                        ob4[t][:, 0:nn, :])
                                    pend[t] = 0
                            else:
                                o = p4p.tile([128, 145], BF, tag="t2os")
                                nc.scalar.copy(out=o[:m, 0:128],
                                               in_=ps[:m, 0:128])
                                nc.vector.tensor_copy(out=o[:m, 128:129],
                                                      in_=ones_sb[:m])
                                nc.vector.tensor_copy(out=o[:m, 129:145],
                                                      in_=ps[:m, 128:144])
                                nc.sync.dma_start(
                                    table2[t][row:row + m, 0:145], o[:m])

            # ---- layer-2 edge phase (x-aggregation, 16-dst groups) ----
            with tc.tile_pool(name="park2", bufs=1) as park2p, \
                 tc.tile_pool(name="eg2", bufs=4) as gp2, \
                 tc.tile_pool(name="ew2", bufs=3) as wp2, \
                 tc.tile_pool(name="es2", bufs=4) as sp2, \
                 tc.tile_pool(name="ef2", bufs=2) as fp2, \
                 tc.tile_pool(name="eps2", bufs=2, space="PSUM") as pp2, \
                 tc.tile_pool(name="ezs2", bufs=2, space="PSUM") as zp2:
                parks2 = {t: park2p.tile([128, NB, 64], F32, tag=f"park2{t}",
                                         name=f"park2{t}") for t in "ab"}
                gidx_sb2 = {}
                sdt2 = {}
                for t in "ab":
                    key = "2" + t
                    ncalls = scheds[key]["TCpad"] // NCH
                    gidx_sb2[t] = fp2.tile([128, ncalls, NCH * 8], I16,
                                           tag=f"gidx{t}", name=f"gidx2{t}",
                                           bufs=1)
                    nc.sync.dma_start(gidx_sb2[t][:],
                                      gidx_d[key].rearrange("c p s -> p c s"))
                    # gather 1 row/rank, then broadcast to round partitions
                    sdraw = fp2.tile([128, NB, 128], BF, tag=f"sdraw{t}",
                                     name=f"sdraw{t}")
                    sdone = 0
                    while sdone < NSLOT:
                        n = min(1024, NSLOT - sdone)
                        nc.gpsimd.dma_gather(
                            sdraw[:, sdone // 128:(sdone + n) // 128, :],
                            table2[t][:, 128:256],
                            sdti_sb[key][:, sdone // 16:(sdone + n) // 16],
                            n, n, 128, elem_step=T2C, queue_num=3)
                        sdone += n
                    cmp8 = fp2.tile([128, NB, 8], BF, tag=f"cmp8{t}",
                                    name=f"cmp8{t}")
                    nc.vector.tensor_copy(out=cmp8[:], in_=sdraw[:, :, 9:17])
                    sdt2[t] = fp2.tile([128, NG2, 8], F32, tag=f"sdt2{t}",
                                       name=f"sdt2{t}", bufs=1)
                    for b in range(8):
                        psr = pp2.tile([128, 160], F32, tag=f"pa{t}",
                                       name="sdtps2").rearrange(
                                           "p (c e) -> p c e", e=8)
                        nc.tensor.matmul(out=psr[:], lhsT=repb_sb[:, b, :],
                                         rhs=cmp8[:], start=True, stop=True)
                        nc.vector.tensor_copy(
                            out=sdt2[t].rearrange("p (c b) e -> p c b e",
                                                  b=8)[:, :, b, :],
                            in_=psr[:])

                st2 = {t: dict(call=-1, G=None, pa=None, p8=None)
                       for t in "ab"}
                qctr2 = [0]

                def do_group2(t, g):
                    key = "2" + t
                    sched = scheds[key]
                    cg = int(sched["Rg"][g] // 8)
                    base = int(sched["cbase"][g])
                    s_ = st2[t]
                    s_["pa"] = pp2.tile([128, 160], F32, tag=f"pa{t}",
                                        name=f"pa2{t}")
                    if g % 8 == 0:
                        s_["p8"] = wp2.tile([128, 8, 128], BF, tag=f"p8{t}",
                                            name=f"p8{t}", bufs=2)
                    pa = s_["pa"]
                    done = 0
                    while done < cg:
                        seg = min(NCH - (base + done) % NCH, cg - done)
                        call = (base + done) // NCH
                        coff = (base + done) % NCH
                        if call != s_["call"]:
                            G = gp2.tile([128, NCH, 256], BF, tag=f"G{t}",
                                         name=f"G2{t}")
                            nc.gpsimd.dma_gather(
                                G[:, :, :], table2[t][:],
                                gidx_sb2[t][:, call, :],
                                NCH * 128, NCH * 128, 256,
                                queue_num=qctr2[0] % 4)
                            qctr2[0] += 1
                            s_["call"] = call
                            s_["G"] = G
                        G = s_["G"]
                        sl = slice(coff, coff + seg)
                        u = sp2.tile([128, NCH, 8], F32, tag=f"u{t}",
                                     name=f"u2{t}")
                        nc.vector.tensor_tensor(
                            out=u[:, :seg, :], in0=G[:, sl, 129:137],
                            in1=sdt2[t][:, g, :][:, None, :].to_broadcast(
                                [128, seg, 8]),
                            op=OP.add)
                        phi = sp2.tile([128, NCH, 8], F32, tag=f"phi{t}",
                                       name=f"phi2{t}")
                        nc.vector.scalar_tensor_tensor(
                            out=phi[:, :seg, :], in0=u[:, :seg, :], scalar=NEG,
                            in1=u[:, :seg, :], op0=OP.mult, op1=OP.max)
                        q = sp2.tile([128, NCH, 8], BF, tag=f"q{t}",
                                     name=f"q2{t}")
                        nc.scalar.activation(out=q[:, :seg, :],
                                             in_=phi[:, :seg, :], func=AF.Exp)
                        Q = wp2.tile([128, NCH, 8, GD2], BF, tag=f"Q{t}",
                                     name=f"Q2{t}")
                        nc.vector.tensor_tensor(
                            out=Q[:, :seg, :, :],
                            in0=q[:, :seg, :, None].to_broadcast(
                                [128, seg, 8, GD2]),
                            in1=sc2_sb[:, None, None, :].to_broadcast(
                                [128, seg, 8, GD2]),
                            op=OP.mult)
                        for s in range(seg):
                            cc = done + s
                            nc.tensor.matmul(
                                out=pa[:, 0:129],
                                lhsT=Q[:, s, :, :],
                                rhs=G[:, coff + s, 0:129],
                                start=(cc == 0), stop=(cc == cg - 1),
                                skip_group_check=True)
                        done += seg

                    if DEBUG and t == "a" and 152 <= g < 160:
                        dpa = sp2.tile([128, 129], F32, tag="dbgpa",
                                       name="dbgpa")
                        nc.vector.tensor_copy(out=dpa[:], in_=pa[:])
                        nc.sync.dma_start(dbg_pa[g - 152], dpa[:])
                    # normalize by 8z, transpose into park8
                    z8 = sp2.tile([128, 1], F32, tag=f"z8{t}", name=f"z82{t}")
                    nc.vector.tensor_scalar(
                        out=z8[:], in0=pa[:, 128:129], scalar1=8.0,
                        scalar2=1e-30, op0=OP.mult, op1=OP.max)
                    rz = sp2.tile([128, 1], F32, tag=f"rz{t}", name=f"rz2{t}")
                    nc.vector.reciprocal(out=rz[:], in_=z8[:])
                    an = sp2.tile([128, 128], BF, tag=f"an{t}", name=f"an2{t}")
                    nc.vector.tensor_tensor(
                        out=an[:], in0=pa[:, 0:128],
                        in1=rz[:].to_broadcast([128, 128]),
                        op=OP.mult)
                    atp = zp2.tile([128, 128], BF, tag="tp",
                                   name=f"atp{t}")
                    nc.tensor.transpose(out=atp[:], in_=an[:],
                                        identity=id_sb[:])
                    nc.vector.tensor_copy(out=s_["p8"][:, g % 8, :], in_=atp[:])
                    if DEBUG and t == "a" and 152 <= g < 160:
                        dan = sp2.tile([128, 128], F32, tag="dbgan",
                                       name="dbgan")
                        nc.vector.tensor_copy(out=dan[:], in_=an[:])
                        nc.sync.dma_start(dbg_an[g - 152], dan[:])
                    if DEBUG and t == "a" and g == 159:
                        dp8 = sp2.tile([128, 8, 128], F32, tag="dbgp8",
                                       name="dbgp8")
                        nc.vector.tensor_copy(out=dp8[:], in_=s_["p8"][:])
                        nc.sync.dma_start(dbg_p8[:], dp8[:])
                    if g % 8 == 7:
                        mi = g // 8
                        B = zp2.tile([64, 128], F32, tag="B", name=f"B{t}")
                        for h in range(HEADS):
                            nc.tensor.matmul(
                                out=B[:],
                                lhsT=w2cat[t][:, 64 * h:64 * (h + 1)],
                                rhs=s_["p8"][:, :, GD2 * h:GD2 * (h + 1)],
                                start=(h == 0), stop=(h == 7),
                                skip_group_check=True)
                        Bs = sp2.tile([64, 128], BF, tag=f"Bs{t}",
                                      name=f"Bs{t}")
                        nc.vector.tensor_copy(out=Bs[:], in_=B[:])
                        if DEBUG and t == "a" and mi == 19:
                            dB = sp2.tile([64, 128], F32, tag="dbgB",
                                          name="dbgB")
                            nc.vector.tensor_copy(out=dB[:], in_=B[:])
                            nc.sync.dma_start(dbg_B[:], dB[:])
                        btp = zp2.tile([128, 128], BF, tag="tp",
                                       name=f"btp{t}")
                        nc.tensor.transpose(out=btp[:, 0:64], in_=Bs[:],
                                            identity=id_sb[0:64, 0:64])
                        nc.vector.tensor_copy(out=parks2[t][:, mi, :],
                                              in_=btp[:, 0:64])

                if DEBUG:
                    nc.sync.dma_start(dbg_sdt2a[:], sdt2["a"][:])
                for g in range(NG2):
                    for t in "ab":
                        do_group2(t, g)
                if DEBUG:
                    nc.sync.dma_start(dbg_parks2a[:], parks2["a"][:])
                    nc.sync.dma_start(dbg_parks2b[:], parks2["b"][:])
                    with tc.tile_pool(name="dbgp", bufs=2) as dbgp:
                        for i in range(2):
                            tt = dbgp.tile([128, 256], BF, tag="dbgt")
                            nc.sync.dma_start(tt[:], table2["a"][i*128:(i+1)*128, :])
                            nc.sync.dma_start(dbg_t2a[i*128:(i+1)*128, :], tt[:])
                for t in "ab":
                    for (c0, nI) in _scatter_calls():
                        nc.gpsimd.dma_scatter_add(
                            acc2[:], parks2[t][:, c0:c0 + nI // 128, :],
                            scat_sb[t][:, c0 * 8:c0 * 8 + nI // 16],
                            nI, nI, 64, queue_num=3)

            if DEBUG:
                with tc.tile_pool(name="dbga", bufs=2) as dbga:
                    for i in range(NB):
                        tt = dbga.tile([128, 64], F32, tag="dbga")
                        nc.sync.dma_start(tt[:], acc2[i*128:(i+1)*128, :])
                        nc.sync.dma_start(dbg_acc2[i*128:(i+1)*128, :], tt[:])

            # ---- classifier ----
            with tc.tile_pool(name="cls", bufs=4) as clsp, \
                 tc.tile_pool(name="clsps", bufs=2, space="PSUM") as clsps:
                def wrc(i, lo, m, h):
                    tps = clsps.tile([64, 128], BF, tag="c_t1")
                    nc.tensor.transpose(out=tps[:, :m], in_=h[:m, :],
                                        identity=id_sb[:m, :m])
                    h3t = clsp.tile([64, 128], BF, tag="c_h3t")
                    nc.scalar.copy(out=h3t[:, :m], in_=tps[:, :m])
                    z1 = clsps.tile([128, 32], F32, tag="c_z1")
                    nc.tensor.matmul(out=z1[:m], lhsT=h3t[:, :m], rhs=wc1_sb[:],
                                     start=True, stop=True)
                    z1s = clsp.tile([128, 32], BF, tag="c_z1s")
                    nc.scalar.activation(out=z1s[:m], in_=z1[:m], func=AF.Relu)
                    t2ps = clsps.tile([32, 128], BF, tag="c_t2")
                    nc.tensor.transpose(out=t2ps[:, :m], in_=z1s[:m, :],
                                        identity=id_sb[:m, :m])
                    z1t = clsp.tile([32, 128], BF, tag="c_z1t")
                    nc.scalar.copy(out=z1t[:, :m], in_=t2ps[:, :m])
                    lg = clsps.tile([128, 2], F32, tag="c_lg")
                    nc.tensor.matmul(out=lg[:m], lhsT=z1t[:, :m], rhs=wc2_sb[:],
                                     start=True, stop=True)
                    lo_ = clsp.tile([128, 2], F32, tag="c_out")
                    nc.vector.tensor_copy(out=lo_[:m], in_=lg[:m])
                    nc.sync.dma_start(out[lo:lo + m, :], lo_[:m])
                elu_combine(acc2[:, :], 64, clsp, wrc)

    nc.compile()
    return nc


# ----------------------------------------------------------------------------
# entry point
# ----------------------------------------------------------------------------

_CACHE = {}


def _prepare(inputs):
    per_core, scheds = _host_prep(inputs)
    key = tuple((k, s["TCpad"], tuple(s["Rg"])) for k, s in sorted(scheds.items()))
    if key not in _CACHE:
        _CACHE.clear()
        _CACHE[key] = _build_nc(scheds)
    return _CACHE[key], per_core


def _run(nc, per_core, **kw):
    from concourse import bass_utils
    return bass_utils.run_bass_kernel_spmd(nc, per_core,
                                           core_ids=list(range(NCORES)), **kw)


def kernel(**inputs):
    nc, per_core = _prepare(inputs)
    res = _run(nc, per_core)
    return np.concatenate([res.results[k]["out"] for k in range(NCORES)], 0)
